# revision 5
# baseline (speedup 1.0000x reference)
"""Multi-head attention (B=2, S=2048, D=768, H=12) on 8 Trainium2 cores.

Sharding: batch x head-group parallel. Core c handles batch b=c//4 and heads
[3*(c%4), 3*(c%4)+3). Each core:
  - computes q/k/v projections for its head slice (column-parallel weights,
    biases folded in via an augmented ones-row on the contraction dim),
  - computes scores^T = K_h Q_h^T in [k, q] layout (k on partitions) so the
    softmax exp output can directly feed the attention*V matmul as lhsT,
  - exp on ScalarE (scale=1/(8)=1/sqrt(d_k) folded into the activation),
    written out UNNORMALIZED to DRAM (host normalizes + transposes),
  - ctx^T = vh_aug^T @ exp(S^T) accumulated on PE; a ones column in vh_aug
    yields the softmax row-sums for free,
  - ctx normalized on-chip (reciprocal + gpsimd partition broadcast),
  - out partial = ctx @ W_o rows (row-parallel); host sums the 4 partials
    per batch and adds b_o.
"""

import numpy as np
import ml_dtypes

import concourse.bacc as bacc
import concourse.mybir as mybir
from concourse.tile import TileContext
from concourse.bass_utils import run_bass_kernel_spmd

B, S, D, H, DK = 2, 2048, 768, 12, 64
HPC = 3           # heads per core
NCORES = 8
KP = 896          # contraction dim 768 + 1 (ones row) padded to 7*128
KT7 = KP // 128   # 7 contraction tiles
ST = S // 128     # 16 sequence tiles
BF = mybir.dt.bfloat16
F32 = mybir.dt.float32

# e_out dtype: fp32 is bit-safer; bf16 halves the dominant DMA traffic.
E_OUT_BF16 = True

_cache = {}


def _build():
    if "nc" in _cache:
        return _cache["nc"]
    nc = bacc.Bacc("TRN2", target_bir_lowering=False, debug=False,
                   num_devices=NCORES)

    xq = nc.dram_tensor("xq", [KP, S], BF, kind="ExternalInput").ap()
    xk = nc.dram_tensor("xk", [KP, S], BF, kind="ExternalInput").ap()
    xv = nc.dram_tensor("xv", [KP, S], BF, kind="ExternalInput").ap()
    wq = nc.dram_tensor("wq", [KP, HPC * DK], BF, kind="ExternalInput").ap()
    wk = nc.dram_tensor("wk", [KP, HPC * DK], BF, kind="ExternalInput").ap()
    wv = nc.dram_tensor("wv", [KP, HPC * DK], BF, kind="ExternalInput").ap()
    wo = nc.dram_tensor("wo", [HPC * 64, D], BF, kind="ExternalInput").ap()
    e_dt = BF if E_OUT_BF16 else F32
    e_out = nc.dram_tensor("e_out", [HPC, ST, 128, S], e_dt,
                           kind="ExternalOutput").ap()
    out_part = nc.dram_tensor("out_part", [ST, 128, D], F32,
                              kind="ExternalOutput").ap()

    with TileContext(nc) as tc:
        with tc.tile_pool(name="persist", bufs=1) as pp:
            # persistent SBUF tensors
            wq_sb = pp.tile([128, KT7, HPC * DK], BF, tag="wq")
            wk_sb = pp.tile([128, KT7, HPC * DK], BF, tag="wk")
            wv_sb = pp.tile([128, KT7, HPC * DK], BF, tag="wv")
            wo_sb = pp.tile([64, HPC, D], BF, tag="wo")
            nc.sync.dma_start(out=wq_sb[:], in_=wq.rearrange("(t p) m -> p t m", p=128))
            nc.sync.dma_start(out=wk_sb[:], in_=wk.rearrange("(t p) m -> p t m", p=128))
            nc.sync.dma_start(out=wv_sb[:], in_=wv.rearrange("(t p) m -> p t m", p=128))
            nc.sync.dma_start(out=wo_sb[:], in_=wo.rearrange("(h p) m -> p h m", p=64))

            qhT = [pp.tile([64, S], BF, name=f"qhT{h}", tag=f"qhT{h}") for h in range(HPC)]
            khT = [pp.tile([64, S], BF, name=f"khT{h}", tag=f"khT{h}") for h in range(HPC)]
            vh_sb = pp.tile([128, ST, HPC, DK + 1], BF, tag="vh")
            ctxT = [pp.tile([64, S], BF, name=f"ctxT{h}", tag=f"ctxT{h}") for h in range(HPC)]
            recip = pp.tile([1, S], F32, tag="recip")
            recip_bc = pp.tile([64, S], F32, tag="recip_bc")

            # ones column for the row-sum trick
            nc.vector.memset(vh_sb[:, :, :, DK:DK + 1], 1.0)

            # ---- Phase 1: projections ----
            with tc.tile_pool(name="xload", bufs=KT7) as xp:
                for name, xap, w_sb, outT in (
                    ("q", xq, wq_sb, qhT),
                    ("k", xk, wk_sb, khT),
                ):
                    xt = []
                    for t in range(KT7):
                        tl = xp.tile([128, S], BF, tag="xl")
                        nc.sync.dma_start(out=tl[:], in_=xap[t * 128:(t + 1) * 128, :])
                        xt.append(tl)
                    with tc.tile_pool(name=f"psA{name}", bufs=4, space="PSUM") as pa:
                        for m in range(2):          # m=0: heads 0,1; m=1: head 2
                            mp = 128 if m == 0 else 64
                            for nq in range(4):
                                ps = pa.tile([128, 512], F32, tag="pA")
                                for t in range(KT7):
                                    nc.tensor.matmul(
                                        ps[:mp, :],
                                        w_sb[:, t, m * 128: m * 128 + mp],
                                        xt[t][:, nq * 512:(nq + 1) * 512],
                                        start=(t == 0), stop=(t == KT7 - 1),
                                    )
                                if m == 0:
                                    nc.vector.tensor_copy(
                                        outT[0][:, nq * 512:(nq + 1) * 512], ps[0:64, :])
                                    nc.vector.tensor_copy(
                                        outT[1][:, nq * 512:(nq + 1) * 512], ps[64:128, :])
                                else:
                                    nc.vector.tensor_copy(
                                        outT[2][:, nq * 512:(nq + 1) * 512], ps[0:64, :])

                # v projection (natural layout, s on partitions)
                xt = []
                for t in range(KT7):
                    tl = xp.tile([128, S], BF, tag="xl")
                    nc.sync.dma_start(out=tl[:], in_=xv[t * 128:(t + 1) * 128, :])
                    xt.append(tl)
                with tc.tile_pool(name="psB", bufs=2, space="PSUM") as pb:
                    for st in range(ST):
                        ps = pb.tile([128, HPC * DK], F32, tag="pB")
                        for t in range(KT7):
                            nc.tensor.matmul(
                                ps[:],
                                xt[t][:, st * 128:(st + 1) * 128],
                                wv_sb[:, t, :],
                                start=(t == 0), stop=(t == KT7 - 1),
                            )
                        nc.vector.tensor_copy(
                            vh_sb[:, st, :, 0:DK],
                            ps[:].rearrange("p (h d) -> p h d", h=HPC))

            # ---- Phase 2: scores^T, exp, AV ----
            with tc.tile_pool(name="esb", bufs=3) as ep, \
                 tc.tile_pool(name="ebf", bufs=3) as ebp, \
                 tc.tile_pool(name="ctxf", bufs=1) as cfp, \
                 tc.tile_pool(name="ps_sc", bufs=4, space="PSUM") as psc, \
                 tc.tile_pool(name="ps_ctx", bufs=1, space="PSUM") as pcx:
                ctx_f = cfp.tile([DK + 1, S], F32, tag="ctxf")
                for h in range(HPC):
                    ctx_ps = pcx.tile([DK + 1, S], F32, tag="ctx")
                    for kt in range(ST):
                        if E_OUT_BF16:
                            ebf = ebp.tile([128, S], BF, tag="ebf")
                        else:
                            esb = ep.tile([128, S], F32, tag="esb")
                            ebf = ebp.tile([128, S], BF, tag="ebf")
                        for nq in range(4):
                            q0 = nq * 512
                            ps = psc.tile([128, 512], F32, tag="sc")
                            nc.tensor.matmul(
                                ps[:],
                                khT[h][:, kt * 128:(kt + 1) * 128],
                                qhT[h][:, q0:q0 + 512],
                                start=True, stop=True,
                            )
                            if E_OUT_BF16:
                                nc.scalar.activation(
                                    ebf[:, q0:q0 + 512], ps[:],
                                    mybir.ActivationFunctionType.Exp, scale=0.125)
                            else:
                                nc.scalar.activation(
                                    esb[:, q0:q0 + 512], ps[:],
                                    mybir.ActivationFunctionType.Exp, scale=0.125)
                                nc.vector.tensor_copy(
                                    ebf[:, q0:q0 + 512], esb[:, q0:q0 + 512])
                            nc.tensor.matmul(
                                ctx_ps[:, q0:q0 + 512],
                                vh_sb[:, kt, h, :],
                                ebf[:, q0:q0 + 512],
                                start=(kt == 0), stop=(kt == ST - 1),
                            )
                        nc.sync.dma_start(
                            out=e_out[h, kt],
                            in_=(ebf[:] if E_OUT_BF16 else esb[:]))
                    # single eviction frees ctx PSUM; recip chain runs off-path
                    nc.vector.tensor_copy(ctx_f[:], ctx_ps[:])
                    nc.vector.reciprocal(recip[0:1, :], ctx_f[DK:DK + 1, :])
                    nc.gpsimd.partition_broadcast(recip_bc[:], recip[0:1, :])
                    nc.vector.tensor_mul(ctxT[h][:], ctx_f[0:DK, :], recip_bc[:])

            # ---- Phase 3: out projection ----
            with tc.tile_pool(name="outst", bufs=2) as op_, \
                 tc.tile_pool(name="ps_out", bufs=2, space="PSUM") as po:
                for st in range(ST):
                    ps = po.tile([128, D], F32, tag="po")
                    for nb, nsz in ((0, 512), (512, 256)):
                        for h in range(HPC):
                            rhs = wo_sb[:, h, nb:nb + nsz]
                            nc.tensor.matmul(
                                ps[:, nb:nb + nsz],
                                ctxT[h][:, st * 128:(st + 1) * 128],
                                rhs,
                                start=(h == 0), stop=(h == HPC - 1),
                            )
                    ot = op_.tile([128, D], F32, tag="ot")
                    nc.vector.tensor_copy(ot[:], ps[:])
                    nc.sync.dma_start(out=out_part[st], in_=ot[:])

    nc.compile()
    _cache["nc"] = nc
    return nc


def _prep(q, k, v, w_q, b_q, w_k, b_k, w_v, b_v, w_o):
    """Build per-core input maps."""
    bf = ml_dtypes.bfloat16
    xs = {}
    for b in range(B):
        for nm, x in (("q", q), ("k", k), ("v", v)):
            xp = np.zeros((KP, S), np.float32)
            xp[:D] = x[b].T
            xp[D] = 1.0
            xs[(nm, b)] = xp.astype(bf)
    in_maps = []
    for c in range(NCORES):
        b = c // (NCORES // B)
        h0 = (c % (NCORES // B)) * HPC
        sl = slice(h0 * DK, (h0 + HPC) * DK)
        m = {"xq": xs[("q", b)], "xk": xs[("k", b)], "xv": xs[("v", b)]}
        for nm, w, bias in (("wq", w_q, b_q), ("wk", w_k, b_k), ("wv", w_v, b_v)):
            wp = np.zeros((KP, HPC * DK), np.float32)
            wp[:D] = w[:, sl]
            wp[D] = bias[sl]
            m[nm] = wp.astype(bf)
        m["wo"] = w_o[sl, :].astype(bf)
        in_maps.append(m)
    return in_maps


def kernel(q, k, v, w_q, b_q, w_k, b_k, w_v, b_v, w_o, b_o):
    nc = _build()
    in_maps = _prep(np.asarray(q, np.float32), np.asarray(k, np.float32),
                    np.asarray(v, np.float32), w_q, b_q, w_k, b_k, w_v, b_v, w_o)
    res = run_bass_kernel_spmd(nc, in_maps, list(range(NCORES)))

    attn = np.empty((B, H, S, S), np.float32)
    out = np.zeros((B, S, D), np.float32)
    for c in range(NCORES):
        b = c // (NCORES // B)
        h0 = (c % (NCORES // B)) * HPC
        E = np.asarray(res.results[c]["e_out"], np.float32).reshape(HPC, S, S)
        for hh in range(HPC):
            Eh = E[hh]                     # [k, q]
            sums = Eh.sum(axis=0)          # [q]
            attn[b, h0 + hh] = (Eh / sums[None, :]).T
        out[b] += np.asarray(res.results[c]["out_part"], np.float32).reshape(S, D)
    out += np.asarray(b_o, np.float32)
    return out, attn


# revision 8
# speedup vs baseline: 1.2648x; 1.2648x over previous
"""Multi-head attention (B=2, S=2048, D=768, H=12) on 8 Trainium2 cores.

Sharding: batch x head-group parallel. Core c handles batch b=c//4 and heads
[3*(c%4), 3*(c%4)+3). Each core:
  - computes q/k/v projections for its head slice (column-parallel weights,
    biases folded in via an augmented ones-row on the contraction dim),
  - computes scores^T = K_h Q_h^T in [k, q] layout (k on partitions) so the
    softmax exp output can directly feed the attention*V matmul as lhsT,
  - exp on ScalarE (scale=1/(8)=1/sqrt(d_k) folded into the activation),
    written out UNNORMALIZED to DRAM (host normalizes + transposes),
  - ctx^T = vh_aug^T @ exp(S^T) accumulated on PE; a ones column in vh_aug
    yields the softmax row-sums for free,
  - ctx normalized on-chip (reciprocal + gpsimd partition broadcast),
  - out partial = ctx @ W_o rows (row-parallel); host sums the 4 partials
    per batch and adds b_o.
"""

import numpy as np
import ml_dtypes

import concourse.bacc as bacc
import concourse.mybir as mybir
from concourse.tile import TileContext
from concourse.bass_utils import run_bass_kernel_spmd

B, S, D, H, DK = 2, 2048, 768, 12, 64
HPC = 3           # heads per core
NCORES = 8
KP = 896          # contraction dim 768 + 1 (ones row) padded to 7*128
KT7 = KP // 128   # 7 contraction tiles
ST = S // 128     # 16 sequence tiles
BF = mybir.dt.bfloat16
F32 = mybir.dt.float32

# e_out dtype: fp32 is bit-safer; bf16 halves the dominant DMA traffic.
E_OUT_BF16 = True

_cache = {}


def _build():
    if "nc" in _cache:
        return _cache["nc"]
    nc = bacc.Bacc("TRN2", target_bir_lowering=False, debug=False,
                   num_devices=NCORES)

    xq = nc.dram_tensor("xq", [KP, S], BF, kind="ExternalInput").ap()
    xk = nc.dram_tensor("xk", [KP, S], BF, kind="ExternalInput").ap()
    xv = nc.dram_tensor("xv", [KP, S], BF, kind="ExternalInput").ap()
    wq = nc.dram_tensor("wq", [KP, HPC * DK], BF, kind="ExternalInput").ap()
    wk = nc.dram_tensor("wk", [KP, HPC * DK], BF, kind="ExternalInput").ap()
    wv = nc.dram_tensor("wv", [KP, HPC * DK], BF, kind="ExternalInput").ap()
    wo = nc.dram_tensor("wo", [HPC * 64, D], BF, kind="ExternalInput").ap()
    e_dt = BF if E_OUT_BF16 else F32
    e_out = nc.dram_tensor("e_out", [HPC, ST, 128, S], e_dt,
                           kind="ExternalOutput").ap()
    out_part = nc.dram_tensor("out_part", [ST, 128, D], F32,
                              kind="ExternalOutput").ap()

    with TileContext(nc) as tc:
        with tc.tile_pool(name="persist", bufs=1) as pp:
            # persistent SBUF tensors
            wq_sb = pp.tile([128, KT7, HPC * DK], BF, tag="wq")
            wk_sb = pp.tile([128, KT7, HPC * DK], BF, tag="wk")
            wv_sb = pp.tile([128, KT7, HPC * DK], BF, tag="wv")
            wo_sb = pp.tile([64, HPC, D], BF, tag="wo")
            nc.sync.dma_start(out=wq_sb[:], in_=wq.rearrange("(t p) m -> p t m", p=128))
            nc.sync.dma_start(out=wk_sb[:], in_=wk.rearrange("(t p) m -> p t m", p=128))
            nc.sync.dma_start(out=wv_sb[:], in_=wv.rearrange("(t p) m -> p t m", p=128))
            nc.sync.dma_start(out=wo_sb[:], in_=wo.rearrange("(h p) m -> p h m", p=64))

            # qhT/khT are duplicated into both partition halves so pairs of
            # score matmuls can run concurrently on PE row groups 0-63/64-127
            qhT = [pp.tile([128, S], BF, name=f"qhT{h}", tag=f"qhT{h}") for h in range(HPC)]
            khT = [pp.tile([128, S], BF, name=f"khT{h}", tag=f"khT{h}") for h in range(HPC)]
            vh_sb = pp.tile([128, ST, HPC, DK + 1], BF, tag="vh")
            ctxT = [pp.tile([64, S], BF, name=f"ctxT{h}", tag=f"ctxT{h}") for h in range(HPC)]
            recip = pp.tile([1, S], F32, tag="recip")
            recip_bc = pp.tile([64, S], F32, tag="recip_bc")

            # ones column for the row-sum trick
            nc.vector.memset(vh_sb[:, :, :, DK:DK + 1], 1.0)

            # ---- Phase 1: projections ----
            with tc.tile_pool(name="xload", bufs=KT7) as xp:
                for name, xap, w_sb, outT in (
                    ("q", xq, wq_sb, qhT),
                    ("k", xk, wk_sb, khT),
                ):
                    xt = []
                    for t in range(KT7):
                        tl = xp.tile([128, S], BF, tag="xl")
                        nc.sync.dma_start(out=tl[:], in_=xap[t * 128:(t + 1) * 128, :])
                        xt.append(tl)
                    with tc.tile_pool(name=f"psA{name}", bufs=4, space="PSUM") as pa:
                        for m in range(2):          # m=0: heads 0,1; m=1: head 2
                            mp = 128 if m == 0 else 64
                            for nq in range(4):
                                ps = pa.tile([128, 512], F32, tag="pA")
                                for t in range(KT7):
                                    nc.tensor.matmul(
                                        ps[:mp, :],
                                        w_sb[:, t, m * 128: m * 128 + mp],
                                        xt[t][:, nq * 512:(nq + 1) * 512],
                                        start=(t == 0), stop=(t == KT7 - 1),
                                    )
                                sl = slice(nq * 512, (nq + 1) * 512)
                                if m == 0:
                                    nc.vector.tensor_copy(outT[0][0:64, sl], ps[0:64, :])
                                    nc.vector.tensor_copy(outT[0][64:128, sl], ps[0:64, :])
                                    nc.vector.tensor_copy(outT[1][0:64, sl], ps[64:128, :])
                                    nc.vector.tensor_copy(outT[1][64:128, sl], ps[64:128, :])
                                else:
                                    nc.vector.tensor_copy(outT[2][0:64, sl], ps[0:64, :])
                                    nc.vector.tensor_copy(outT[2][64:128, sl], ps[0:64, :])

                # v projection (natural layout, s on partitions)
                xt = []
                for t in range(KT7):
                    tl = xp.tile([128, S], BF, tag="xl")
                    nc.sync.dma_start(out=tl[:], in_=xv[t * 128:(t + 1) * 128, :])
                    xt.append(tl)
                with tc.tile_pool(name="psB", bufs=2, space="PSUM") as pb:
                    for st in range(ST):
                        ps = pb.tile([128, HPC * DK], F32, tag="pB")
                        for t in range(KT7):
                            nc.tensor.matmul(
                                ps[:],
                                xt[t][:, st * 128:(st + 1) * 128],
                                wv_sb[:, t, :],
                                start=(t == 0), stop=(t == KT7 - 1),
                            )
                        nc.vector.tensor_copy(
                            vh_sb[:, st, :, 0:DK],
                            ps[:].rearrange("p (h d) -> p h d", h=HPC))

            # ---- Phase 2: scores^T, exp, AV ----
            # Phase 2: per head, per pair of k-tiles: the two score matmuls
            # (K=64 each) run concurrently on PE row groups 0-63 / 64-127 into
            # one [128,1024] PSUM tile, one exp covers both, AV accumulates.
            with tc.tile_pool(name="ebf", bufs=3) as ebp, \
                 tc.tile_pool(name="ctxf", bufs=1) as cfp, \
                 tc.tile_pool(name="ps_sc", bufs=2, space="PSUM") as psc, \
                 tc.tile_pool(name="ps_ctx", bufs=1, space="PSUM") as pcx:
                ctx_f = cfp.tile([DK + 1, S], F32, tag="ctxf")
                for h in range(HPC):
                    ctx_ps = pcx.tile([DK + 1, S], F32, tag="ctx")
                    for kp in range(ST // 2):
                        kt0, kt1 = 2 * kp, 2 * kp + 1
                        # [128 part, 4 nq, 2 kt, 512 q]
                        ebf = ebp.tile([128, 4, 2, 512], BF, tag="ebf")
                        for nq in range(4):
                            q0 = nq * 512
                            ps = psc.tile([128, 1024], F32, tag="sc")
                            nc.tensor.matmul(
                                ps[:, 0:512],
                                khT[h][0:64, kt0 * 128:(kt0 + 1) * 128],
                                qhT[h][0:64, q0:q0 + 512],
                                start=True, stop=True,
                            )
                            nc.tensor.matmul(
                                ps[:, 512:1024],
                                khT[h][64:128, kt1 * 128:(kt1 + 1) * 128],
                                qhT[h][64:128, q0:q0 + 512],
                                start=True, stop=True,
                            )
                            nc.scalar.activation(
                                ebf[:, nq], ps[:].rearrange("p (a b) -> p a b", a=2),
                                mybir.ActivationFunctionType.Exp, scale=0.125)
                            nc.tensor.matmul(
                                ctx_ps[:, q0:q0 + 512],
                                vh_sb[:, kt0, h, :],
                                ebf[:, nq, 0, :],
                                start=(kt0 == 0), stop=False,
                            )
                            nc.tensor.matmul(
                                ctx_ps[:, q0:q0 + 512],
                                vh_sb[:, kt1, h, :],
                                ebf[:, nq, 1, :],
                                start=False, stop=(kt1 == ST - 1),
                            )
                        nc.sync.dma_start(out=e_out[h, kt0], in_=ebf[:, :, 0, :])
                        nc.sync.dma_start(out=e_out[h, kt1], in_=ebf[:, :, 1, :])
                    # single eviction frees ctx PSUM; recip chain runs off-path
                    nc.vector.tensor_copy(ctx_f[:], ctx_ps[:])
                    nc.vector.reciprocal(recip[0:1, :], ctx_f[DK:DK + 1, :])
                    nc.gpsimd.partition_broadcast(recip_bc[:], recip[0:1, :])
                    nc.vector.tensor_mul(ctxT[h][:], ctx_f[0:DK, :], recip_bc[:])

            # ---- Phase 3: out projection ----
            with tc.tile_pool(name="outst", bufs=2) as op_, \
                 tc.tile_pool(name="ps_out", bufs=2, space="PSUM") as po:
                for st in range(ST):
                    ps = po.tile([128, D], F32, tag="po")
                    for nb, nsz in ((0, 512), (512, 256)):
                        for h in range(HPC):
                            rhs = wo_sb[:, h, nb:nb + nsz]
                            nc.tensor.matmul(
                                ps[:, nb:nb + nsz],
                                ctxT[h][:, st * 128:(st + 1) * 128],
                                rhs,
                                start=(h == 0), stop=(h == HPC - 1),
                            )
                    ot = op_.tile([128, D], F32, tag="ot")
                    nc.vector.tensor_copy(ot[:], ps[:])
                    nc.sync.dma_start(out=out_part[st], in_=ot[:])

    nc.compile()
    _cache["nc"] = nc
    return nc


def _prep(q, k, v, w_q, b_q, w_k, b_k, w_v, b_v, w_o):
    """Build per-core input maps."""
    bf = ml_dtypes.bfloat16
    xs = {}
    for b in range(B):
        for nm, x in (("q", q), ("k", k), ("v", v)):
            xp = np.zeros((KP, S), np.float32)
            xp[:D] = x[b].T
            xp[D] = 1.0
            xs[(nm, b)] = xp.astype(bf)
    in_maps = []
    for c in range(NCORES):
        b = c // (NCORES // B)
        h0 = (c % (NCORES // B)) * HPC
        sl = slice(h0 * DK, (h0 + HPC) * DK)
        m = {"xq": xs[("q", b)], "xk": xs[("k", b)], "xv": xs[("v", b)]}
        for nm, w, bias in (("wq", w_q, b_q), ("wk", w_k, b_k), ("wv", w_v, b_v)):
            wp = np.zeros((KP, HPC * DK), np.float32)
            wp[:D] = w[:, sl]
            wp[D] = bias[sl]
            m[nm] = wp.astype(bf)
        m["wo"] = w_o[sl, :].astype(bf)
        in_maps.append(m)
    return in_maps


def kernel(q, k, v, w_q, b_q, w_k, b_k, w_v, b_v, w_o, b_o):
    nc = _build()
    in_maps = _prep(np.asarray(q, np.float32), np.asarray(k, np.float32),
                    np.asarray(v, np.float32), w_q, b_q, w_k, b_k, w_v, b_v, w_o)
    res = run_bass_kernel_spmd(nc, in_maps, list(range(NCORES)))

    attn = np.empty((B, H, S, S), np.float32)
    out = np.zeros((B, S, D), np.float32)
    for c in range(NCORES):
        b = c // (NCORES // B)
        h0 = (c % (NCORES // B)) * HPC
        E = np.asarray(res.results[c]["e_out"], np.float32).reshape(HPC, S, S)
        for hh in range(HPC):
            Eh = E[hh]                     # [k, q]
            sums = Eh.sum(axis=0)          # [q]
            attn[b, h0 + hh] = (Eh / sums[None, :]).T
        out[b] += np.asarray(res.results[c]["out_part"], np.float32).reshape(S, D)
    out += np.asarray(b_o, np.float32)
    return out, attn


# revision 10
# speedup vs baseline: 1.2789x; 1.0112x over previous
"""Multi-head attention (B=2, S=2048, D=768, H=12) on 8 Trainium2 cores.

Sharding: batch x head-group parallel. Core c handles batch b=c//4 and heads
[3*(c%4), 3*(c%4)+3). Each core:
  - computes q/k/v projections for its head slice (column-parallel weights,
    biases folded in via an augmented ones-row on the contraction dim),
  - computes scores^T = K_h Q_h^T in [k, q] layout (k on partitions) so the
    softmax exp output can directly feed the attention*V matmul as lhsT,
  - exp on ScalarE (scale=1/(8)=1/sqrt(d_k) folded into the activation),
    written out UNNORMALIZED to DRAM (host normalizes + transposes),
  - ctx^T = vh_aug^T @ exp(S^T) accumulated on PE; a ones column in vh_aug
    yields the softmax row-sums for free,
  - ctx normalized on-chip (reciprocal + gpsimd partition broadcast),
  - out partial = ctx @ W_o rows (row-parallel); host sums the 4 partials
    per batch and adds b_o.
"""

import numpy as np
import ml_dtypes

import concourse.bacc as bacc
import concourse.mybir as mybir
from concourse.tile import TileContext
from concourse.bass_utils import run_bass_kernel_spmd

B, S, D, H, DK = 2, 2048, 768, 12, 64
HPC = 3           # heads per core
NCORES = 8
KP = 896          # contraction dim 768 + 1 (ones row) padded to 7*128
KT7 = KP // 128   # 7 contraction tiles
ST = S // 128     # 16 sequence tiles
BF = mybir.dt.bfloat16
F32 = mybir.dt.float32

# e_out dtype: fp32 is bit-safer; bf16 halves the dominant DMA traffic.
E_OUT_BF16 = True

_cache = {}


def _build():
    if "nc" in _cache:
        return _cache["nc"]
    nc = bacc.Bacc("TRN2", target_bir_lowering=False, debug=False,
                   num_devices=NCORES)

    xq = nc.dram_tensor("xq", [KP, S], BF, kind="ExternalInput").ap()
    xk = nc.dram_tensor("xk", [KP, S], BF, kind="ExternalInput").ap()
    xv = nc.dram_tensor("xv", [KP, S], BF, kind="ExternalInput").ap()
    wq = nc.dram_tensor("wq", [KP, HPC * DK], BF, kind="ExternalInput").ap()
    wk = nc.dram_tensor("wk", [KP, HPC * DK], BF, kind="ExternalInput").ap()
    wv = nc.dram_tensor("wv", [KP, HPC * DK], BF, kind="ExternalInput").ap()
    wo = nc.dram_tensor("wo", [HPC * 64, D], BF, kind="ExternalInput").ap()
    e_dt = BF if E_OUT_BF16 else F32
    e_out = nc.dram_tensor("e_out", [HPC, ST, 128, S], e_dt,
                           kind="ExternalOutput").ap()
    out_part = nc.dram_tensor("out_part", [ST, 128, D], F32,
                              kind="ExternalOutput").ap()

    with TileContext(nc) as tc:
        with tc.tile_pool(name="persist", bufs=1) as pp:
            # persistent SBUF tensors
            wq_sb = pp.tile([128, KT7, HPC * DK], BF, tag="wq")
            wk_sb = pp.tile([128, KT7, HPC * DK], BF, tag="wk")
            wv_sb = pp.tile([128, KT7, HPC * DK], BF, tag="wv")
            wo_sb = pp.tile([64, HPC, D], BF, tag="wo")
            nc.sync.dma_start(out=wq_sb[:], in_=wq.rearrange("(t p) m -> p t m", p=128))
            nc.sync.dma_start(out=wk_sb[:], in_=wk.rearrange("(t p) m -> p t m", p=128))
            nc.sync.dma_start(out=wv_sb[:], in_=wv.rearrange("(t p) m -> p t m", p=128))
            nc.sync.dma_start(out=wo_sb[:], in_=wo.rearrange("(h p) m -> p h m", p=64))

            # qhT/khT are duplicated into both partition halves so pairs of
            # score matmuls can run concurrently on PE row groups 0-63/64-127
            qhT = [pp.tile([128, S], BF, name=f"qhT{h}", tag=f"qhT{h}") for h in range(HPC)]
            khT = [pp.tile([128, S], BF, name=f"khT{h}", tag=f"khT{h}") for h in range(HPC)]
            vh_sb = pp.tile([128, ST, HPC, DK + 1], BF, tag="vh")
            ctxT = [pp.tile([64, S], BF, name=f"ctxT{h}", tag=f"ctxT{h}") for h in range(HPC)]
            recip = pp.tile([1, S], F32, tag="recip")
            recip_bc = pp.tile([64, S], F32, tag="recip_bc")

            # ones column for the row-sum trick
            nc.vector.memset(vh_sb[:, :, :, DK:DK + 1], 1.0)

            # ---- Phase 1: projections (v first so q/k finish adjacent to
            # phase 2's first score matmuls) ----
            with tc.tile_pool(name="xload", bufs=KT7) as xp, \
                 tc.tile_pool(name="ps1", bufs=8, space="PSUM") as p1:
                # v projection (natural layout, s on partitions)
                xt = []
                for t in range(KT7):
                    tl = xp.tile([128, S], BF, tag="xl")
                    nc.sync.dma_start(out=tl[:], in_=xv[t * 128:(t + 1) * 128, :])
                    xt.append(tl)
                for st in range(ST):
                    ps = p1.tile([128, 512], F32, tag="p1")
                    for t in range(KT7):
                        nc.tensor.matmul(
                            ps[:, 0:HPC * DK],
                            xt[t][:, st * 128:(st + 1) * 128],
                            wv_sb[:, t, :],
                            start=(t == 0), stop=(t == KT7 - 1),
                        )
                    nc.vector.tensor_copy(
                        vh_sb[:, st, :, 0:DK],
                        ps[:, 0:HPC * DK].rearrange("p (h d) -> p h d", h=HPC))

                for name, xap, w_sb, outT in (
                    ("k", xk, wk_sb, khT),
                    ("q", xq, wq_sb, qhT),
                ):
                    xt = []
                    for t in range(KT7):
                        tl = xp.tile([128, S], BF, tag="xl")
                        nc.sync.dma_start(out=tl[:], in_=xap[t * 128:(t + 1) * 128, :])
                        xt.append(tl)
                    for m in range(2):          # m=0: heads 0,1; m=1: head 2
                        mp = 128 if m == 0 else 64
                        for nq in range(4):
                            ps = p1.tile([128, 512], F32, tag="p1")
                            for t in range(KT7):
                                nc.tensor.matmul(
                                    ps[:mp, :],
                                    w_sb[:, t, m * 128: m * 128 + mp],
                                    xt[t][:, nq * 512:(nq + 1) * 512],
                                    start=(t == 0), stop=(t == KT7 - 1),
                                )
                            sl = slice(nq * 512, (nq + 1) * 512)
                            if m == 0:
                                nc.vector.tensor_copy(outT[0][0:64, sl], ps[0:64, :])
                                nc.vector.tensor_copy(outT[0][64:128, sl], ps[0:64, :])
                                nc.vector.tensor_copy(outT[1][0:64, sl], ps[64:128, :])
                                nc.vector.tensor_copy(outT[1][64:128, sl], ps[64:128, :])
                            else:
                                nc.vector.tensor_copy(outT[2][0:64, sl], ps[0:64, :])
                                nc.vector.tensor_copy(outT[2][64:128, sl], ps[0:64, :])

            # ---- Phase 2: scores^T, exp, AV ----
            # Phase 2: per head, per pair of k-tiles: the two score matmuls
            # (K=64 each) run concurrently on PE row groups 0-63 / 64-127 into
            # one [128,1024] PSUM tile, one exp covers both, AV accumulates.
            with tc.tile_pool(name="ebf", bufs=3) as ebp, \
                 tc.tile_pool(name="ctxf", bufs=1) as cfp, \
                 tc.tile_pool(name="ps_sc", bufs=2, space="PSUM") as psc, \
                 tc.tile_pool(name="ps_ctx", bufs=1, space="PSUM") as pcx:
                ctx_f = cfp.tile([DK + 1, S], F32, tag="ctxf")
                for h in range(HPC):
                    ctx_ps = pcx.tile([DK + 1, S], F32, tag="ctx")
                    for kp in range(ST // 2):
                        kt0, kt1 = 2 * kp, 2 * kp + 1
                        # [128 part, 4 nq, 2 kt, 512 q]
                        ebf = ebp.tile([128, 4, 2, 512], BF, tag="ebf")
                        for nq in range(4):
                            q0 = nq * 512
                            ps = psc.tile([128, 1024], F32, tag="sc")
                            nc.tensor.matmul(
                                ps[:, 0:512],
                                khT[h][0:64, kt0 * 128:(kt0 + 1) * 128],
                                qhT[h][0:64, q0:q0 + 512],
                                start=True, stop=True,
                            )
                            nc.tensor.matmul(
                                ps[:, 512:1024],
                                khT[h][64:128, kt1 * 128:(kt1 + 1) * 128],
                                qhT[h][64:128, q0:q0 + 512],
                                start=True, stop=True,
                            )
                            nc.scalar.activation(
                                ebf[:, nq], ps[:].rearrange("p (a b) -> p a b", a=2),
                                mybir.ActivationFunctionType.Exp, scale=0.125)
                            nc.tensor.matmul(
                                ctx_ps[:, q0:q0 + 512],
                                vh_sb[:, kt0, h, :],
                                ebf[:, nq, 0, :],
                                start=(kt0 == 0), stop=False,
                            )
                            nc.tensor.matmul(
                                ctx_ps[:, q0:q0 + 512],
                                vh_sb[:, kt1, h, :],
                                ebf[:, nq, 1, :],
                                start=False, stop=(kt1 == ST - 1),
                            )
                        nc.sync.dma_start(out=e_out[h, kt0], in_=ebf[:, :, 0, :])
                        nc.sync.dma_start(out=e_out[h, kt1], in_=ebf[:, :, 1, :])
                    # single eviction frees ctx PSUM; recip chain runs off-path.
                    # 1/s via exp(-ln(s)) on ScalarE: same table set as Exp,
                    # ~4us vs 13us for the single-partition DVE reciprocal.
                    nc.vector.tensor_copy(ctx_f[:], ctx_ps[:])
                    nc.scalar.activation(recip[0:1, :], ctx_f[DK:DK + 1, :],
                                         mybir.ActivationFunctionType.Ln)
                    nc.scalar.activation(recip[0:1, :], recip[0:1, :],
                                         mybir.ActivationFunctionType.Exp,
                                         scale=-1.0)
                    nc.gpsimd.partition_broadcast(recip_bc[:], recip[0:1, :])
                    nc.vector.tensor_mul(ctxT[h][:], ctx_f[0:DK, :], recip_bc[:])

            # ---- Phase 3: out projection ----
            with tc.tile_pool(name="outst", bufs=2) as op_, \
                 tc.tile_pool(name="ps_out", bufs=2, space="PSUM") as po:
                for st in range(ST):
                    ps = po.tile([128, D], F32, tag="po")
                    for nb, nsz in ((0, 512), (512, 256)):
                        for h in range(HPC):
                            rhs = wo_sb[:, h, nb:nb + nsz]
                            nc.tensor.matmul(
                                ps[:, nb:nb + nsz],
                                ctxT[h][:, st * 128:(st + 1) * 128],
                                rhs,
                                start=(h == 0), stop=(h == HPC - 1),
                            )
                    ot = op_.tile([128, D], F32, tag="ot")
                    nc.vector.tensor_copy(ot[:], ps[:])
                    nc.sync.dma_start(out=out_part[st], in_=ot[:])

    nc.compile()
    _cache["nc"] = nc
    return nc


def _prep(q, k, v, w_q, b_q, w_k, b_k, w_v, b_v, w_o):
    """Build per-core input maps."""
    bf = ml_dtypes.bfloat16
    xs = {}
    for b in range(B):
        for nm, x in (("q", q), ("k", k), ("v", v)):
            xp = np.zeros((KP, S), np.float32)
            xp[:D] = x[b].T
            xp[D] = 1.0
            xs[(nm, b)] = xp.astype(bf)
    in_maps = []
    for c in range(NCORES):
        b = c // (NCORES // B)
        h0 = (c % (NCORES // B)) * HPC
        sl = slice(h0 * DK, (h0 + HPC) * DK)
        m = {"xq": xs[("q", b)], "xk": xs[("k", b)], "xv": xs[("v", b)]}
        for nm, w, bias in (("wq", w_q, b_q), ("wk", w_k, b_k), ("wv", w_v, b_v)):
            wp = np.zeros((KP, HPC * DK), np.float32)
            wp[:D] = w[:, sl]
            wp[D] = bias[sl]
            m[nm] = wp.astype(bf)
        m["wo"] = w_o[sl, :].astype(bf)
        in_maps.append(m)
    return in_maps


def kernel(q, k, v, w_q, b_q, w_k, b_k, w_v, b_v, w_o, b_o):
    nc = _build()
    in_maps = _prep(np.asarray(q, np.float32), np.asarray(k, np.float32),
                    np.asarray(v, np.float32), w_q, b_q, w_k, b_k, w_v, b_v, w_o)
    res = run_bass_kernel_spmd(nc, in_maps, list(range(NCORES)))

    attn = np.empty((B, H, S, S), np.float32)
    out = np.zeros((B, S, D), np.float32)
    for c in range(NCORES):
        b = c // (NCORES // B)
        h0 = (c % (NCORES // B)) * HPC
        E = np.asarray(res.results[c]["e_out"], np.float32).reshape(HPC, S, S)
        for hh in range(HPC):
            Eh = E[hh]                     # [k, q]
            sums = Eh.sum(axis=0)          # [q]
            attn[b, h0 + hh] = (Eh / sums[None, :]).T
        out[b] += np.asarray(res.results[c]["out_part"], np.float32).reshape(S, D)
    out += np.asarray(b_o, np.float32)
    return out, attn
